# revision 37
# baseline (speedup 1.0000x reference)
"""Trainium2 Bass kernel for BitwiseTasNetBlock (v2 restructure).

Data-parallel over batch: 8 cores x 2 batch items. All matmul I/O is bf16
(weights are +-1, exact; activations round to ~0.4% -- tolerance is 2e-2).
bf16 weights enable FWL (2x faster LDWEIGHTS, hidden under 512-col matmuls).

Per layer:
  A: w1 1x1 conv (bf16 matmuls, kt-outer/chunk-inner for weight reuse),
     PReLU fused into 1024-col psum->sbuf ACT drains, bn_stats on DVE.
  sync1: per-channel (sum,sumsq) AllGather (4.6us floor vs AR 9.7us) +
     local DVE reduce. While it is in flight the PE runs dconv matmuls:
     the first RAW_ITERS chunk-iters drain psum->scratch ungated (split
     Act/DVE), everything later drains fused (scale=s1) once s1 lands.
  B: depthwise dilated conv as diagonal matmuls (3 taps, psum accum),
     BN1+PReLU2 fused in the drain via positive-homogeneous folding of
     the causal-pad boundary (3 bias variants at c==0), bn_stats2.
  sync2: AllGather -> BN2 folded into w2 weights + bias3.
  C: w2 1x1 conv + bias; drains feed next layer's xbuf (bf16). Last layer
     adds the f32 residual (Pool+DVE) and DMAs out.
P2 overwrites Y1 in place (reverse chunk order makes Y1[c] dead after
dconv(c)).
"""
import sys

sys.path.insert(0, "/opt/trn_rl_repo")
import numpy as np
import ml_dtypes

BF16 = np.dtype(ml_dtypes.bfloat16)

L, CB, D, KTAP = 4, 256, 512, 3
B, T = 16, 4096
EPS = 1e-5
NCORES = 8
BLOC = B // NCORES          # 2 batch items per core
CHUNK = 512
NCT = T // CHUNK            # 8 time chunks per batch item
PAD = 16
DOFF = PAD
YCOLS = PAD + T
NGLOB = float(B * T)

# chunk-iter order: reverse time within each batch item
KORD = [(b, c) for b in range(BLOC) for c in range(NCT - 1, -1, -1)]
# phase-A groups: adjacent (c, c-1) pairs, ybuf-contiguous
GROUPS = [(KORD[2 * g], KORD[2 * g + 1]) for g in range(len(KORD) // 2)]
RAW_MAX = 5                 # raw scratch pair-groups
RAW_PER_LAYER = [5, 5, 5, 5]  # dconv pair-groups drained raw (covers sync1)

# packed per-channel vector indices
(V_B1, V_BD, V_G1, V_BE1, V_G2, V_BE2, V_WSA, V_WS12, V_WS2) = range(9)
NVEC = 9

_cache = {}


def _build(a1_vals, a2_vals):
    import concourse.bass as bass
    import concourse.tile as tile
    from concourse import bacc, mybir

    f32 = mybir.dt.float32
    bf16 = mybir.dt.bfloat16
    Alu = mybir.AluOpType
    Act = mybir.ActivationFunctionType

    nc = bacc.Bacc(None, target_bir_lowering=False, debug=False, num_devices=NCORES)

    xinb_d = nc.dram_tensor("xinb", [BLOC, CB, T], bf16, kind="ExternalInput")
    xinf_d = nc.dram_tensor("xinf", [BLOC, CB, T], f32, kind="ExternalInput")
    w1t_d = nc.dram_tensor("w1t", [L, 128, 2, 4, 128], bf16, kind="ExternalInput")
    w2t_d = nc.dram_tensor("w2t", [L, 128, 4, 2, 128], bf16, kind="ExternalInput")
    diag_d = nc.dram_tensor("diag", [L, 128, 3, 4, 128], bf16, kind="ExternalInput")
    vecs_d = nc.dram_tensor("vecs", [128, L, NVEC, 4], f32, kind="ExternalInput")
    b2_d = nc.dram_tensor("b2v", [128, L, 2], f32, kind="ExternalInput")
    out_d = nc.dram_tensor("out", [BLOC, CB, T], f32, kind="ExternalOutput")

    # shared-output AllGather buffers: one per sync point (monotonic use)
    NSYNC = 2 * L
    gouts = [
        nc.dram_tensor(f"gout{s}", [NCORES, 128, 8], f32, kind="Internal",
                       addr_space="Shared")
        for s in range(NSYNC)
    ]
    galn_d = nc.dram_tensor("galn", [NCORES, 128, 1], f32, kind="Internal",
                            addr_space="Shared")

    with tile.TileContext(nc) as tc:
        with (
            tc.tile_pool(name="persist", bufs=1) as pp,
            tc.tile_pool(name="wts", bufs=2) as wp,
            tc.tile_pool(name="raw", bufs=1) as rawp,
            tc.tile_pool(name="stats", bufs=2) as statsp,
            tc.tile_pool(name="vec", bufs=12) as vecp,
            tc.tile_pool(name="stage", bufs=4) as stagep,
            tc.tile_pool(name="psa", bufs=2, space="PSUM") as psa,
            tc.tile_pool(name="psbc", bufs=2, space="PSUM") as psbc,
            tc.tile_pool(name="dram", bufs=2, space="DRAM") as dramp,
        ):
            # ---- persistent SBUF ----
            xbuf = [pp.tile([128, BLOC, T], bf16, tag=f"xb{kt}", name=f"xb{kt}")
                    for kt in range(2)]
            ybuf = [pp.tile([128, BLOC, YCOLS], bf16, tag=f"yb{ct}", name=f"yb{ct}")
                    for ct in range(4)]
            rawt = [rawp.tile([128, 2, CHUNK], bf16, tag=f"raw{i}", name=f"raw{i}")
                    for i in range(RAW_MAX * 4)]

            vecs_sb = pp.tile([128, L, NVEC, 4], f32, tag="vecs")
            b2_sb = pp.tile([128, L, 2], f32, tag="b2sb")
            epsc = pp.tile([128, 1], f32, tag="epsc")
            alnsb = pp.tile([128, 1], f32, tag="alnsb")

            nc.sync.dma_start(vecs_sb[:], vecs_d[:])
            nc.sync.dma_start(b2_sb[:], b2_d[:])
            nc.vector.memset(epsc[:], EPS)
            nc.vector.memset(alnsb[:], 0.0)

            # zero causal pads (bf16 zeros)
            for ct in range(4):
                for b in range(BLOC):
                    nc.vector.memset(ybuf[ct][:, b, 0:PAD], 0.0)

            # x load: chunked DMAs in REV processing order so the first
            # phase-A group's data lands first; gpsimd queue issues are cheap
            for (b, c) in KORD:
                for kt in range(2):
                    eng = nc.sync
                    eng.dma_start(
                        xbuf[kt][:, b, CHUNK * c:CHUNK * (c + 1)],
                        xinb_d[b, 128 * kt:128 * (kt + 1), CHUNK * c:CHUNK * (c + 1)],
                    )

            # startup-alignment collective (absorbs core skew during x load)
            aln_in = dramp.tile([128, 1], f32, tag="alnin", name="alnin")
            nc.sync.dma_start(aln_in[:], alnsb[:])
            nc.gpsimd.collective_compute(
                "AllGather", Alu.bypass,
                replica_groups=[list(range(NCORES))],
                ins=[aln_in[:].opt()], outs=[galn_d[:].opt()],
            )

            # layer-0 weights
            w1sb = wp.tile([128, 2, 4, 128], bf16, tag="w1sb", name="w1sb_0")
            w2sb = wp.tile([128, 4, 2, 128], bf16, tag="w2sb", name="w2sb_0")
            dgsb = wp.tile([128, 3, 4, 128], bf16, tag="dgsb", name="dgsb_0")
            nc.gpsimd.dma_start(w1sb[:], w1t_d[0])
            nc.gpsimd.dma_start(w2sb[:], w2t_d[0])
            nc.gpsimd.dma_start(dgsb[:], diag_d[0])

            def ag_sync(s, csb):
                """AllGather csb [128,8] across cores; return gath sbuf tile."""
                cin = dramp.tile([128, 8], f32, tag="cin", name=f"cin{s}")
                nc.sync.dma_start(cin[:], csb[:])
                nc.gpsimd.collective_compute(
                    "AllGather", Alu.bypass,
                    replica_groups=[list(range(NCORES))],
                    ins=[cin[:].opt()], outs=[gouts[s][:].opt()],
                )
                gath = vecp.tile([128, 8, 8], f32, tag="gath", bufs=2)
                nc.sync.dma_start(
                    gath[:], gouts[s][:].rearrange("r p v -> p r v")
                )
                return gath

            def stats_pre(st):
                """Local (sum, sumsq) [128, 4+4] from bn_stats triples."""
                st3 = st[:].rearrange("p ct k (h s) -> p ct (k h) s", s=3)
                means = st3[:, :, :, 1]
                m2s = st3[:, :, :, 2]
                csb = vecp.tile([128, 8], f32, tag="csb")
                sums_r = vecp.tile([128, 4], f32, tag="sums_r")
                nc.vector.tensor_reduce(
                    sums_r[:], means, axis=mybir.AxisListType.X, op=Alu.add
                )
                nc.vector.tensor_scalar(
                    csb[:, 0:4], sums_r[:], float(CHUNK // 2), None, op0=Alu.mult
                )
                msq = vecp.tile([128, 4, 32], f32, tag="msq", bufs=2)
                nc.vector.tensor_mul(msq[:], means, means)
                nc.vector.scalar_tensor_tensor(
                    msq[:], msq[:], float(CHUNK // 2), m2s,
                    op0=Alu.mult, op1=Alu.add,
                )
                nc.vector.tensor_reduce(
                    csb[:, 4:8], msq[:], axis=mybir.AxisListType.X, op=Alu.add
                )
                return csb

            def stats_post(i, gath, g_idx, be_idx):
                """Global s, t from gathered per-core (sum, sumsq)."""
                gsum = vecp.tile([128, 8], f32, tag="gsum")
                nc.vector.tensor_reduce(
                    gsum[:], gath[:].rearrange("p r v -> p v r"),
                    axis=mybir.AxisListType.X, op=Alu.add,
                )
                mv8 = vecp.tile([128, 8], f32, tag="mv8")
                nc.vector.tensor_scalar(
                    mv8[:], gsum[:], 1.0 / NGLOB, None, op0=Alu.mult
                )
                mean4 = mv8[:, 0:4]
                var4 = vecp.tile([128, 4], f32, tag="var4")
                nc.vector.tensor_mul(var4[:], mean4, mean4)
                nc.vector.tensor_sub(var4[:], mv8[:, 4:8], var4[:])
                std4 = vecp.tile([128, 4], f32, tag="std4")
                nc.scalar.activation(std4[:], var4[:], Act.Sqrt, bias=epsc[:], scale=1.0)
                rstd4 = vecp.tile([128, 4], f32, tag="rstd4")
                nc.vector.reciprocal(rstd4[:], std4[:])
                s4 = vecp.tile([128, 4], f32, tag="s4")
                nc.vector.tensor_mul(s4[:], rstd4[:], vecs_sb[:, i, g_idx, :])
                t4 = vecp.tile([128, 4], f32, tag="t4")
                nc.vector.tensor_mul(t4[:], mean4, s4[:])
                nc.vector.tensor_sub(t4[:], vecs_sb[:, i, be_idx, :], t4[:])
                return s4, t4

            for i in range(L):
                dil = 2 ** i
                a1i = float(a1_vals[i])
                a2i = float(a2_vals[i])

                st1 = statsp.tile([128, 4, 16, 6], f32, tag="st1", name=f"st1_{i}")
                st2 = statsp.tile([128, 4, 16, 6], f32, tag="st2", name=f"st2_{i}")

                # ---- Phase A: w1 matmuls + PReLU drains + stats1 ----
                # kt-inner-pair ordering reuses each weight block for 2 chunks;
                # paired 1024-col drains halve the Act instruction count
                for g, ((b, c_hi), (_, c_lo)) in enumerate(GROUPS):
                    for mt in range(4):
                        ps = psa.tile([128, 2, 512], f32, tag="psa",
                                      name=f"psA_{i}_{g}_{mt}")
                        for kt in range(2):
                            for j, c in enumerate((c_lo, c_hi)):
                                nc.tensor.matmul(
                                    ps[:, j, :],
                                    w1sb[:, kt, mt, :],
                                    xbuf[kt][:, b, CHUNK * c:CHUNK * (c + 1)],
                                    start=(kt == 0), stop=(kt == 1),
                                )
                        ysl = ybuf[mt][:, b,
                                       DOFF + CHUNK * c_lo:DOFF + CHUNK * (c_hi + 1)]
                        nc.scalar.activation(
                            ysl, ps[:], Act.Prelu,
                            bias=vecs_sb[:, i, V_B1, mt:mt + 1], scale=1.0,
                            alpha=a1i,
                        )
                        for j, c in enumerate((c_lo, c_hi)):
                            nc.vector.bn_stats(
                                st1[:, mt, b * NCT + c, :],
                                ybuf[mt][:, b, DOFF + CHUNK * c:DOFF + CHUNK * (c + 1)],
                            )

                # weight prefetch for next layer (cheap, on gpsimd queue)
                if i + 1 < L:
                    w1nx = wp.tile([128, 2, 4, 128], bf16, tag="w1sb",
                                   name=f"w1sb_{i+1}")
                    w2nx = wp.tile([128, 4, 2, 128], bf16, tag="w2sb",
                                   name=f"w2sb_{i+1}")
                    dgnx = wp.tile([128, 3, 4, 128], bf16, tag="dgsb",
                                   name=f"dgsb_{i+1}")
                    nc.gpsimd.dma_start(w1nx[:], w1t_d[i + 1])
                    nc.gpsimd.dma_start(w2nx[:], w2t_d[i + 1])
                    nc.gpsimd.dma_start(dgnx[:], diag_d[i + 1])

                # ---- dconv matmul emission (PE): one [128,2,512] psum
                # tile per (chunk-pair, ct); tap-outer ordering reuses each
                # diagonal weight block across both chunks
                def emit_dconv_pair(g, ct):
                    (b, c_hi), (_, c_lo) = GROUPS[g]
                    # alternate pools: psa is idle during phase B, so using it
                    # for every other tile doubles the effective ring depth
                    if (g * 4 + ct) % 2 == 0:
                        ps = psa.tile([128, 2, 512], f32, tag="psa",
                                      name=f"psB_{i}_{g}_{ct}")
                    else:
                        ps = psbc.tile([128, 2, 512], f32, tag="psbc",
                                       name=f"psB_{i}_{g}_{ct}")
                    for j in range(KTAP):
                        for jj, c in enumerate((c_lo, c_hi)):
                            off = DOFF + CHUNK * c - (2 - j) * dil
                            nc.tensor.matmul(
                                ps[:, jj, :],
                                dgsb[:, j, ct, :],
                                ybuf[ct][:, b, off:off + CHUNK],
                                start=(j == 0), stop=(j == KTAP - 1),
                            )
                    return ps

                # sync1 launch (DVE pre-reduce emitted after stats1)
                csb1 = stats_pre(st1)
                gath1 = ag_sync(2 * i, csb1)

                RAW_GRPS = RAW_PER_LAYER[i]
                raw_ps = {}
                for g in range(RAW_GRPS):
                    for ct in range(4):
                        ps = emit_dconv_pair(g, ct)
                        slot = rawt[g * 4 + ct]
                        # raw drain, ungated by s1; alternate Act/DVE
                        if ct % 2 == 0:
                            nc.scalar.activation(slot[:], ps[:], Act.Copy)
                        else:
                            nc.vector.tensor_copy(slot[:], ps[:])
                        raw_ps[(g, ct)] = slot

                # post-AG math for sync1
                s1, t1 = stats_post(i, gath1, V_G1, V_BE1)
                b2a4 = vecp.tile([128, 4], f32, tag="b2a4")
                b2b4 = vecp.tile([128, 4], f32, tag="b2b4")
                b2c4 = vecp.tile([128, 4], f32, tag="b2c4")
                for dst, widx in ((b2a4, V_WSA), (b2b4, V_WS12), (b2c4, V_WS2)):
                    nc.vector.tensor_mul(dst[:], t1[:], vecs_sb[:, i, widx, :])
                    nc.vector.tensor_add(dst[:], dst[:], vecs_sb[:, i, V_BD, :])

                # ---- Phase B drains: fused first (they free psum slots
                # and restart the PE the moment s1 lands), raw fixups after.
                # One 1024-col ACT per (pair, ct); c==0 boundary overridden.
                for g in list(range(RAW_GRPS, len(GROUPS))) + list(range(RAW_GRPS)):
                    (b, c_hi), (_, c_lo) = GROUPS[g]
                    po = DOFF + CHUNK * c_lo
                    for ct in range(4):
                        p2sl = ybuf[ct][:, b, po:po + 2 * CHUNK]
                        if g < RAW_GRPS:
                            src_ = raw_ps[(g, ct)][:]
                        else:
                            ps = emit_dconv_pair(g, ct)
                            src_ = ps[:]
                        nc.scalar.activation(
                            p2sl, src_, Act.Prelu,
                            bias=b2a4[:, ct:ct + 1], scale=s1[:, ct:ct + 1],
                            alpha=a2i,
                        )
                        if c_lo == 0:
                            nc.scalar.activation(
                                ybuf[ct][:, b, po:po + dil],
                                src_[:, 0, 0:dil], Act.Prelu,
                                bias=b2c4[:, ct:ct + 1], scale=s1[:, ct:ct + 1],
                                alpha=a2i,
                            )
                            nc.scalar.activation(
                                ybuf[ct][:, b, po + dil:po + 2 * dil],
                                src_[:, 0, dil:2 * dil], Act.Prelu,
                                bias=b2b4[:, ct:ct + 1], scale=s1[:, ct:ct + 1],
                                alpha=a2i,
                            )
                        for c in (c_lo, c_hi):
                            nc.vector.bn_stats(
                                st2[:, ct, b * NCT + c, :],
                                ybuf[ct][:, b,
                                         DOFF + CHUNK * c:DOFF + CHUNK * (c + 1)],
                            )

                # ---- sync2 + BN2 fold into w2 ----
                csb2 = stats_pre(st2)
                gath2 = ag_sync(2 * i + 1, csb2)
                s2, t2 = stats_post(i, gath2, V_G2, V_BE2)

                w2r = wp.tile([128, 4, 2, 128], bf16, tag="w2r", name=f"w2r_{i}")
                for kt in range(4):
                    nc.vector.tensor_scalar(
                        w2r[:, kt, :, :], w2sb[:, kt, :, :], s2[:, kt:kt + 1], None,
                        op0=Alu.mult,
                    )
                # keep the PE (and HAM clock gate) warm through the sync2
                # window: chained dummy matmuls into a psb slot, released by a
                # 1-column read
                # gate the warm chain on the LAST phase-B ybuf write (the
                # final raw-group fixup), so it runs exactly in the sync2 gap
                warm = psbc.tile([128, 2, 512], f32, tag="psbc", name=f"warm_{i}")
                (wb, _), (_, wc) = GROUPS[-1]
                for wi in range(28):
                    nc.tensor.matmul(
                        warm[:, 0, :], dgsb[:, 0, 0, :],
                        ybuf[0][:, wb, DOFF + CHUNK * wc:DOFF + CHUNK * (wc + 1)],
                        start=True, stop=True,
                    )
                wjunk = vecp.tile([128, 1], f32, tag="wjunk")
                nc.scalar.activation(wjunk[:], warm[:, 0, 0:1], Act.Copy)

                # bias3 = w2 @ t2 + b2 (the s2 folding cancels exactly), using
                # RAW w2 weights so it does not wait on the w2r fold. Output
                # lands in a psa-ring slot: no extra psum bank needed.
                t24 = vecp.tile([128, 4, 2], bf16, tag="t24")
                nc.vector.tensor_copy(t24[:, :, 0], t2[:])
                nc.vector.tensor_copy(t24[:, :, 1], t2[:])
                psb3 = psa.tile([128, 2, 512], f32, tag="psa", name=f"psd_{i}")
                for mt in range(2):
                    for kt in range(4):
                        nc.tensor.matmul(
                            psb3[:, 0, 2 * mt:2 * mt + 2],
                            w2sb[:, kt, mt, :],
                            t24[:, kt, :],
                            start=(kt == 0), stop=(kt == 3),
                        )
                bias3 = []
                for mt in range(2):
                    b3 = vecp.tile([128, 1], f32, tag="b3")
                    nc.scalar.activation(
                        b3[:], psb3[:, 0, 2 * mt:2 * mt + 1], Act.Identity,
                        bias=b2_sb[:, i, mt:mt + 1], scale=1.0,
                    )
                    bias3.append(b3)

                # ---- Phase C: w2 matmuls; drains feed xbuf (or output) ----
                if i == L - 1:
                    # prefetch residual chunks (f32) for the last layer
                    rstage = {}
                    for g, ((b, c_hi), (_, c_lo)) in enumerate(GROUPS):
                        for mt in range(2):
                            rt = stagep.tile([128, 2 * CHUNK], f32, tag="rst",
                                             bufs=4, name=f"rst_{g}_{mt}")
                            nc.sync.dma_start(
                                rt[:],
                                xinf_d[b, 128 * mt:128 * (mt + 1),
                                       CHUNK * c_lo:CHUNK * (c_hi + 1)],
                            )
                            rstage[(g, mt)] = rt

                for g, ((b, c_hi), (_, c_lo)) in enumerate(GROUPS):
                    for mt in range(2):
                        ps = psbc.tile([128, 2, 512], f32, tag="psbc",
                                       name=f"psC_{i}_{g}_{mt}")
                        for kt in range(4):
                            for j, c in enumerate((c_lo, c_hi)):
                                nc.tensor.matmul(
                                    ps[:, j, :],
                                    w2r[:, kt, mt, :],
                                    ybuf[kt][:, b,
                                             DOFF + CHUNK * c:DOFF + CHUNK * (c + 1)],
                                    start=(kt == 0), stop=(kt == 3),
                                )
                        if i < L - 1:
                            if (g + mt) % 4 == 3:
                                nc.vector.tensor_scalar(
                                    xbuf[mt][:, b,
                                             CHUNK * c_lo:CHUNK * (c_hi + 1)],
                                    ps[:], bias3[mt][:], None, op0=Alu.add,
                                )
                            else:
                                nc.scalar.activation(
                                    xbuf[mt][:, b, CHUNK * c_lo:CHUNK * (c_hi + 1)],
                                    ps[:], Act.Identity,
                                    bias=bias3[mt][:], scale=1.0,
                                )
                        else:
                            ot = stagep.tile([128, 2 * CHUNK], f32, tag="ost",
                                             bufs=4, name=f"ost_{g}_{mt}")
                            nc.scalar.activation(
                                ot[:], ps[:], Act.Identity,
                                bias=bias3[mt][:], scale=1.0,
                            )
                            rt = rstage[(g, mt)]
                            # residual add: split DVE / Pool
                            eng = nc.gpsimd if (g + mt) % 4 == 3 else nc.vector
                            eng.tensor_add(ot[:], ot[:], rt[:])
                            nc.sync.dma_start(
                                out_d[b, 128 * mt:128 * (mt + 1),
                                      CHUNK * c_lo:CHUNK * (c_hi + 1)],
                                ot[:],
                            )

                w1sb, w2sb, dgsb = (w1nx, w2nx, dgnx) if i + 1 < L else (None,) * 3

    nc.compile()
    return nc


def _prep_inputs(x, w1, b1, a1, g1, be1, wd, bd, a2, g2, be2, w2, b2):
    """Host-side packing. All weights binarized via sign()."""
    w1b = np.sign(w1[..., 0]).astype(np.float32)        # [L, D, CB]
    wdb = np.sign(wd[..., 0, :]).astype(np.float32)     # [L, D, K]
    w2b = np.sign(w2[..., 0]).astype(np.float32)        # [L, CB, D]

    w1t = np.empty((L, 128, 2, 4, 128), BF16)
    w2t = np.empty((L, 128, 4, 2, 128), BF16)
    diag = np.zeros((L, 128, 3, 4, 128), BF16)
    eye = np.eye(128, dtype=np.float32)
    for i in range(L):
        for kt in range(2):
            for mt in range(4):
                blk = w1b[i, 128 * mt:128 * (mt + 1), 128 * kt:128 * (kt + 1)]
                w1t[i, :, kt, mt, :] = blk.T.astype(BF16)
        for kt in range(4):
            for mt in range(2):
                blk = w2b[i, 128 * mt:128 * (mt + 1), 128 * kt:128 * (kt + 1)]
                w2t[i, :, kt, mt, :] = blk.T.astype(BF16)
        for j in range(3):
            for ct in range(4):
                diag[i, :, j, ct, :] = (
                    eye * wdb[i, 128 * ct:128 * (ct + 1), j][None, :]
                ).astype(BF16)

    wsa = wdb.sum(-1)
    ws12 = wdb[:, :, 1] + wdb[:, :, 2]
    ws2 = wdb[:, :, 2]
    vec_list = [b1, bd, g1, be1, g2, be2, wsa, ws12, ws2]
    vecs = np.empty((128, L, NVEC, 4), np.float32)
    for v, arr in enumerate(vec_list):
        vecs[:, :, v, :] = np.asarray(arr).reshape(L, 4, 128).transpose(2, 0, 1)
    b2v = np.asarray(b2).reshape(L, 2, 128).transpose(2, 0, 1).astype(np.float32)
    return w1t, w2t, diag, vecs, b2v


def kernel(**inputs):
    from concourse.bass_utils import run_bass_kernel_spmd

    inputs = {k: np.asarray(v, dtype=np.float32) for k, v in inputs.items()}
    x = inputs["x"]
    w1t, w2t, diag, vecs, b2v = _prep_inputs(**inputs)
    xb = x.astype(BF16)

    key = "nc"
    if key not in _cache:
        _cache[key] = _build(inputs["a1"], inputs["a2"])
    nc = _cache[key]

    in_maps = []
    for i in range(NCORES):
        in_maps.append({
            "xinb": np.ascontiguousarray(xb[BLOC * i:BLOC * (i + 1)]),
            "xinf": np.ascontiguousarray(x[BLOC * i:BLOC * (i + 1)]),
            "w1t": w1t, "w2t": w2t, "diag": diag, "vecs": vecs, "b2v": b2v,
        })
    import os
    trace = bool(int(os.environ.get("BASS_KERNEL_TRACE", "0")))
    res = run_bass_kernel_spmd(
        nc, in_maps, core_ids=list(range(NCORES)), trace=trace,
    )
    _cache["last_results"] = res
    out = np.empty((B, CB, T), np.float32)
    for i in range(NCORES):
        out[BLOC * i:BLOC * (i + 1)] = res.results[i]["out"]
    return out
